# revision 13
# baseline (speedup 1.0000x reference)
"""GAT block (graph attention) Bass/Tile kernel for Trainium2, 8 NeuronCores.

Full-input contract: kernel(x=(8,2048,128), W=(128,64), a=(128,1)) -> (8,2048,64).
Sharding: data-parallel over batch — one batch element per core, W/a replicated.

Per-core math (N=2048, Fin=128, Fout=64):
  h  = x @ W                               (N, Fout)
  s1 = h @ a[:64, 0],  s2 = h @ a[64:, 0]  (N,)
  e[i, j]   = leakyrelu(s1[i] + s2[j], 0.2)
  att       = softmax(e, axis=0)  (normalize over i for each column j)
  out       = leakyrelu(att @ h, 0.2)

Implementation notes:
  * attention matrix kept transposed: Pt[j, i] = exp(lrelu(s1[i] + s2[j])).
    leakyrelu tiles are split across ACT and DVE(+GPSIMD) to balance engines;
    the exp runs on ACT (Prelu/parametric_relu shares the exp activation-table
    set so there are no table reloads) with accum_out giving the softmax
    denominator for free.
  * No max-subtraction: |s1+s2| <~ 15 so exp is far from fp32 overflow; this
    matches jax softmax to fp32 rounding.
  * setup matmuls run in float32r (single-pass) — fp32 matmuls on TRN2 are
    two-pass (LOW/HIGH) and twice the cost.  h and both score projections
    come from one stationary [W | W@a1 | W@a2] per x-tile.
  * out is accumulated transposed (hpT[f, i] in 4 PSUM banks, one per
    512-wide i-chunk) so the 64 bf16 matmuls overlap the ACT/DVE stream
    tile-by-tile; the host un-transposes the (64, 2048) result.
"""

import numpy as np
from contextlib import ExitStack

import concourse.bass as bass
import concourse.mybir as mybir
import concourse.tile as tile
from concourse import bacc
from concourse._compat import with_exitstack
from concourse.bass_utils import run_bass_kernel_spmd
from concourse.masks import make_identity

F32 = mybir.dt.float32
F32R = mybir.dt.float32r
BF16 = mybir.dt.bfloat16
AF = mybir.ActivationFunctionType
ALU = mybir.AluOpType

N = 2048
FIN = 128
FOUT = 64
P = 128
T = N // P          # 16 row tiles
NC = N // 512       # 4 i-chunks for the output accumulation
NEG_SLOPE = 0.2
N_CORES = 8

# leakyrelu-tile engine assignment (exp always runs on ACT; non-ACT tiles
# compute z*0.2 on GPSIMD and the max on DVE)
ACT_TILES = {0, 1, 2}


@with_exitstack
def _gat_body(ctx: ExitStack, tc: tile.TileContext, x, w, a, out):
    nc = tc.nc

    const = ctx.enter_context(tc.tile_pool(name="const", bufs=1))
    xin = ctx.enter_context(tc.tile_pool(name="xin", bufs=8))
    lpool = ctx.enter_context(tc.tile_pool(name="lrelu", bufs=4))
    dpool = ctx.enter_context(tc.tile_pool(name="denoms", bufs=2 * T))

    # ---- constants / persistent tiles ----
    ident = const.tile([P, P], F32)
    make_identity(nc, ident)
    w_raw = const.tile([FIN, FOUT], F32)
    nc.sync.dma_start(w_raw[:], w)
    a_raw = const.tile([FOUT, 2], F32)  # [:,0]=a1, [:,1]=a2
    nc.sync.dma_start(a_raw[:, 0:1], a[0:FOUT, :])
    nc.sync.dma_start(a_raw[:, 1:2], a[FOUT:, :])
    acol = const.tile([FOUT, 2], F32R)
    nc.vector.tensor_copy(acol[:], a_raw[:])
    ones_raw = const.tile([1, P], F32)
    nc.vector.memset(ones_raw[:], 1.0)
    ones_row = const.tile([1, P], F32R)
    nc.vector.tensor_copy(ones_row[:], ones_raw[:])

    xT = const.tile([P, T, P], F32R)        # x transposed: [k, t, n]
    hs12 = const.tile([P, T, FOUT + 2], F32)  # [h | s1 s2 cols] per tile
    hs_bf = const.tile([P, T, FOUT], BF16)  # h/denom in bf16
    wsa = const.tile([FIN, FOUT + 2], F32R)  # [W | W@a1 | W@a2]
    s1b = const.tile([P, N], F32)           # s1 broadcast along partitions
    srow = const.tile([2, N], F32R)         # [s1, s2] as rows
    p_all = const.tile([P, T, N], BF16)     # attention numerator, transposed
    o_sb = const.tile([FOUT, N], F32)       # output transposed

    with tc.tile_pool(name="ps_tr", bufs=3, space="PSUM") as ps_tr, \
         tc.tile_pool(name="ps_mm", bufs=2, space="PSUM") as ps_mm:
        # wsa = [W | W @ [a1, a2]]  (wa via wT = W.T, contraction over f)
        nc.vector.tensor_copy(wsa[:, 0:FOUT], w_raw[:])
        ps_wT = ps_mm.tile([FOUT, FIN], F32, tag="ps_h")
        nc.tensor.transpose(ps_wT[:], w_raw[:], ident[:])
        wT = const.tile([FOUT, FIN], F32R)
        nc.vector.tensor_copy(wT[:], ps_wT[:])
        ps_wa = ps_mm.tile([FIN, 2], F32, tag="ps_r")
        nc.tensor.matmul(ps_wa[:], lhsT=wT[:], rhs=acol[:], start=True, stop=True)
        nc.vector.tensor_copy(wsa[:, FOUT:], ps_wa[:])

        # per x-tile: load (2 DMA queues), transpose, [h|s12] matmul, srow matmul
        for t in range(T):
            xn = xin.tile([P, FIN], F32, tag="xn")
            dma_eng = (nc.sync, nc.gpsimd, nc.scalar)[t % 3]
            dma_eng.dma_start(xn[:], x[t * P:(t + 1) * P, :])
            psT = ps_tr.tile([P, P], F32, tag="ps_t")
            nc.tensor.transpose(psT[:], xn[:], ident[:])
            # f32->f32r rounding copy on ACT (idle during setup)
            nc.scalar.copy(xT[:, t, :], psT[:])

        for t in range(T):
            ps_h = ps_mm.tile([P, FOUT + 2], F32, tag="ps_h")
            nc.tensor.matmul(ps_h[:], lhsT=xT[:, t, :], rhs=wsa[:],
                             start=True, stop=True)
            nc.vector.tensor_copy(hs12[:, t, :], ps_h[:])

            # score rows for this tile: (2, 128) = wa.T @ x_t.T
            ps_r = ps_mm.tile([2, P], F32, tag="ps_r")
            nc.tensor.matmul(ps_r[:], lhsT=wsa[:, FOUT:], rhs=xT[:, t, :],
                             start=True, stop=True)
            nc.vector.tensor_copy(srow[:, t * P:(t + 1) * P], ps_r[:])

            # broadcast s1 row chunk once its 4 tiles are in
            if t % 4 == 3:
                c = t // 4
                sl = slice(c * 512, (c + 1) * 512)
                ps_b = ps_mm.tile([P, 512], F32, tag="ps_h")
                nc.tensor.matmul(ps_b[:], lhsT=ones_row[:], rhs=srow[0:1, sl],
                                 start=True, stop=True)
                nc.vector.tensor_copy(s1b[:, sl], ps_b[:])

    # setup PSUM pools released; output accumulators take the banks
    ps_out = ctx.enter_context(tc.tile_pool(name="ps_out", bufs=1, space="PSUM"))
    hp_ps = [ps_out.tile([FOUT, 512], F32, tag=f"hp{c}", name=f"hp{c}")
             for c in range(NC)]

    # 0.2*s1b and 0.2*s2col so GPSIMD's z02 is a single-op tensor_scalar
    s1b02 = const.tile([P, N], F32)
    nc.vector.tensor_scalar_mul(s1b02[:], s1b[:], NEG_SLOPE)
    s2c02 = const.tile([P, T], F32)
    nc.vector.tensor_scalar_mul(s2c02[:], hs12[:, :, FOUT + 1], NEG_SLOPE)

    # ---- main: per j-tile lrelu -> exp(+denom) -> scale h -> accumulate out ----
    for t in range(T):
        s2c = hs12[:, t, FOUT + 1:FOUT + 2]
        if t in ACT_TILES:
            l_t = lpool.tile([P, N], F32, tag="l")
            nc.scalar.activation(l_t[:], s1b[:], AF.Prelu,
                                 bias=s2c, scale=1.0, alpha=NEG_SLOPE)
        else:
            z02 = lpool.tile([P, N], F32, tag="z02")
            nc.gpsimd.tensor_scalar(z02[:], s1b02[:], s2c02[:, t:t + 1], None,
                                    op0=ALU.add)
            l_t = lpool.tile([P, N], F32, tag="l")
            nc.vector.scalar_tensor_tensor(l_t[:], in0=s1b[:], scalar=s2c,
                                           in1=z02[:], op0=ALU.add, op1=ALU.max)

        den_t = dpool.tile([P, 1], F32, tag="den")
        nc.scalar.activation(p_all[:, t, :], l_t[:], AF.Exp, accum_out=den_t[:])

        rden_t = dpool.tile([P, 1], F32, tag="rden")
        nc.vector.reciprocal(rden_t[:], den_t[:])
        nc.vector.tensor_scalar_mul(hs_bf[:, t, :], hs12[:, t, 0:FOUT], rden_t[:])

        for c in range(NC):
            nc.tensor.matmul(hp_ps[c][:], lhsT=hs_bf[:, t, :],
                             rhs=p_all[:, t, c * 512:(c + 1) * 512],
                             start=(t == 0), stop=(t == T - 1))

    # ---- epilogue: leakyrelu on ACT straight from PSUM, DMA out transposed ----
    for c in range(NC):
        sl = slice(c * 512, (c + 1) * 512)
        nc.scalar.activation(o_sb[:, sl], hp_ps[c][:], AF.Prelu,
                             bias=0.0, scale=1.0, alpha=NEG_SLOPE)
        nc.sync.dma_start(out[:, sl], o_sb[:, sl])


_NC_CACHE = {}


def _build_nc():
    if "nc" in _NC_CACHE:
        return _NC_CACHE["nc"]
    nc = bacc.Bacc("TRN2", target_bir_lowering=False, debug=False)
    x = nc.dram_tensor("x", (N, FIN), F32, kind="ExternalInput").ap()
    w = nc.dram_tensor("w", (FIN, FOUT), F32, kind="ExternalInput").ap()
    a = nc.dram_tensor("a", (2 * FOUT, 1), F32, kind="ExternalInput").ap()
    # transposed output; the host un-transposes
    out = nc.dram_tensor("out", (FOUT, N), F32, kind="ExternalOutput").ap()
    with tile.TileContext(nc) as tc:
        _gat_body(tc, x, w, a, out)
    nc.compile()
    _NC_CACHE["nc"] = nc
    return nc


def kernel(x, W, a):
    x = np.ascontiguousarray(np.asarray(x), dtype=np.float32)
    W = np.ascontiguousarray(np.asarray(W), dtype=np.float32)
    a = np.ascontiguousarray(np.asarray(a), dtype=np.float32)
    assert x.shape == (N_CORES, N, FIN), x.shape
    nc = _build_nc()
    in_maps = [{"x": x[c], "w": W, "a": a} for c in range(N_CORES)]
    res = run_bass_kernel_spmd(nc, in_maps, core_ids=list(range(N_CORES)))
    return np.stack([res.results[c]["out"].T.copy() for c in range(N_CORES)], axis=0)


# revision 14
# speedup vs baseline: 4.6352x; 4.6352x over previous
"""GAT block (graph attention) Bass/Tile kernel for Trainium2, 8 NeuronCores.

Full-input contract: kernel(x=(8,2048,128), W=(128,64), a=(128,1)) -> (8,2048,64).
Sharding: data-parallel over batch — one batch element per core, W/a replicated.

Per-core math (N=2048, Fin=128, Fout=64):
  h  = x @ W                               (N, Fout)
  s1 = h @ a[:64, 0],  s2 = h @ a[64:, 0]  (N,)
  e[i, j]   = leakyrelu(s1[i] + s2[j], 0.2)
  att       = softmax(e, axis=0)  (normalize over i for each column j)
  out       = leakyrelu(att @ h, 0.2)

Implementation notes:
  * attention matrix kept transposed: Pt[j, i] = exp(lrelu(s1[i] + s2[j])).
    leakyrelu tiles are split across ACT and DVE(+GPSIMD) to balance engines;
    the exp runs on ACT (Prelu/parametric_relu shares the exp activation-table
    set so there are no table reloads) with accum_out giving the softmax
    denominator for free.
  * No max-subtraction: |s1+s2| <~ 15 so exp is far from fp32 overflow; this
    matches jax softmax to fp32 rounding.
  * setup matmuls run in float32r (single-pass) — fp32 matmuls on TRN2 are
    two-pass (LOW/HIGH) and twice the cost.  h and both score projections
    come from one stationary [W | W@a1 | W@a2] per x-tile.
  * out is accumulated transposed (hpT[f, i] in 4 PSUM banks, one per
    512-wide i-chunk) so the 64 bf16 matmuls overlap the ACT/DVE stream
    tile-by-tile; the host un-transposes the (64, 2048) result.
"""

import numpy as np
from contextlib import ExitStack

import concourse.bass as bass
import concourse.mybir as mybir
import concourse.tile as tile
from concourse import bacc
from concourse._compat import with_exitstack
from concourse.bass_utils import run_bass_kernel_spmd
from concourse.masks import make_identity

F32 = mybir.dt.float32
F32R = mybir.dt.float32r
BF16 = mybir.dt.bfloat16
AF = mybir.ActivationFunctionType
ALU = mybir.AluOpType

N = 2048
FIN = 128
FOUT = 64
P = 128
T = N // P          # 16 row tiles
NC = N // 512       # 4 i-chunks for the output accumulation
NEG_SLOPE = 0.2
N_CORES = 8

# leakyrelu-tile engine assignment (exp always runs on ACT; non-ACT tiles
# compute z*0.2 on GPSIMD and the max on DVE)
ACT_TILES = {0, 1, 2}


@with_exitstack
def _gat_body(ctx: ExitStack, tc: tile.TileContext, x, w, a, out):
    nc = tc.nc

    const = ctx.enter_context(tc.tile_pool(name="const", bufs=1))
    xin = ctx.enter_context(tc.tile_pool(name="xin", bufs=8))
    lpool = ctx.enter_context(tc.tile_pool(name="lrelu", bufs=4))
    dpool = ctx.enter_context(tc.tile_pool(name="denoms", bufs=2 * T))

    # ---- constants / persistent tiles ----
    ident = const.tile([P, P], F32)
    make_identity(nc, ident)
    w_raw = const.tile([FIN, FOUT], F32)
    nc.sync.dma_start(w_raw[:], w)
    a_raw = const.tile([FOUT, 2], F32)  # [:,0]=a1, [:,1]=a2
    nc.sync.dma_start(a_raw[:, 0:1], a[0:FOUT, :])
    nc.sync.dma_start(a_raw[:, 1:2], a[FOUT:, :])
    acol = const.tile([FOUT, 2], F32R)
    nc.vector.tensor_copy(acol[:], a_raw[:])
    ones_raw = const.tile([1, P], F32)
    nc.vector.memset(ones_raw[:], 1.0)
    ones_row = const.tile([1, P], F32R)
    nc.vector.tensor_copy(ones_row[:], ones_raw[:])

    xT = const.tile([P, T, P], F32R)        # x transposed: [k, t, n]
    hs12 = const.tile([P, T, FOUT + 2], F32)  # [h | s1 s2 cols] per tile
    hs_bf = const.tile([P, T, FOUT], BF16)  # h/denom in bf16
    wsa = const.tile([FIN, FOUT + 2], F32R)  # [W | W@a1 | W@a2]
    s1b = const.tile([P, N], F32)           # s1 broadcast along partitions
    srow = const.tile([2, N], F32R)         # [s1, s2] as rows
    p_all = const.tile([P, T, N], BF16)     # attention numerator, transposed
    o_sb = const.tile([FOUT, N], F32)       # output transposed

    with tc.tile_pool(name="ps_tr", bufs=3, space="PSUM") as ps_tr, \
         tc.tile_pool(name="ps_mm", bufs=2, space="PSUM") as ps_mm:
        # wsa = [W | W @ [a1, a2]]  (wa via wT = W.T, contraction over f)
        nc.vector.tensor_copy(wsa[:, 0:FOUT], w_raw[:])
        ps_wT = ps_mm.tile([FOUT, FIN], F32, tag="ps_h")
        nc.tensor.transpose(ps_wT[:], w_raw[:], ident[:])
        wT = const.tile([FOUT, FIN], F32R)
        nc.vector.tensor_copy(wT[:], ps_wT[:])
        ps_wa = ps_mm.tile([FIN, 2], F32, tag="ps_r")
        nc.tensor.matmul(ps_wa[:], lhsT=wT[:], rhs=acol[:], start=True, stop=True)
        nc.vector.tensor_copy(wsa[:, FOUT:], ps_wa[:])

        # per x-tile: load (2 DMA queues), transpose, [h|s12] matmul, srow matmul
        for t in range(T):
            xn = xin.tile([P, FIN], F32, tag="xn")
            dma_eng = (nc.sync, nc.gpsimd, nc.scalar)[t % 3]
            dma_eng.dma_start(xn[:], x[t * P:(t + 1) * P, :])
            psT = ps_tr.tile([P, P], F32, tag="ps_t")
            nc.tensor.transpose(psT[:], xn[:], ident[:])
            # f32->f32r rounding copy on ACT (idle during setup)
            nc.scalar.copy(xT[:, t, :], psT[:])

        for t in range(T):
            ps_h = ps_mm.tile([P, FOUT + 2], F32, tag="ps_h")
            nc.tensor.matmul(ps_h[:], lhsT=xT[:, t, :], rhs=wsa[:],
                             start=True, stop=True)
            nc.vector.tensor_copy(hs12[:, t, :], ps_h[:])

            # score rows for this tile: (2, 128) = wa.T @ x_t.T
            ps_r = ps_mm.tile([2, P], F32, tag="ps_r")
            nc.tensor.matmul(ps_r[:], lhsT=wsa[:, FOUT:], rhs=xT[:, t, :],
                             start=True, stop=True)
            nc.vector.tensor_copy(srow[:, t * P:(t + 1) * P], ps_r[:])

            # broadcast s1 row chunk once its 4 tiles are in
            if t % 4 == 3:
                c = t // 4
                sl = slice(c * 512, (c + 1) * 512)
                ps_b = ps_mm.tile([P, 512], F32, tag="ps_h")
                nc.tensor.matmul(ps_b[:], lhsT=ones_row[:], rhs=srow[0:1, sl],
                                 start=True, stop=True)
                nc.vector.tensor_copy(s1b[:, sl], ps_b[:])

    # setup PSUM pools released; output accumulators take the banks
    ps_out = ctx.enter_context(tc.tile_pool(name="ps_out", bufs=1, space="PSUM"))
    hp_ps = [ps_out.tile([FOUT, 512], F32, tag=f"hp{c}", name=f"hp{c}")
             for c in range(NC)]

    # ---- main: per j-tile lrelu -> exp(+denom) -> scale h -> accumulate out ----
    for t in range(T):
        s2c = hs12[:, t, FOUT + 1:FOUT + 2]
        if t in ACT_TILES:
            l_t = lpool.tile([P, N], F32, tag="l")
            nc.scalar.activation(l_t[:], s1b[:], AF.Prelu,
                                 bias=s2c, scale=1.0, alpha=NEG_SLOPE)
        else:
            z02 = lpool.tile([P, N], F32, tag="z02")
            nc.gpsimd.tensor_scalar(z02[:], s1b[:], s2c, NEG_SLOPE,
                                    op0=ALU.add, op1=ALU.mult)
            l_t = lpool.tile([P, N], F32, tag="l")
            nc.vector.scalar_tensor_tensor(l_t[:], in0=s1b[:], scalar=s2c,
                                           in1=z02[:], op0=ALU.add, op1=ALU.max)

        den_t = dpool.tile([P, 1], F32, tag="den")
        nc.scalar.activation(p_all[:, t, :], l_t[:], AF.Exp, accum_out=den_t[:])

        rden_t = dpool.tile([P, 1], F32, tag="rden")
        nc.vector.reciprocal(rden_t[:], den_t[:])
        nc.vector.tensor_scalar_mul(hs_bf[:, t, :], hs12[:, t, 0:FOUT], rden_t[:])

        for c in range(NC):
            nc.tensor.matmul(hp_ps[c][:], lhsT=hs_bf[:, t, :],
                             rhs=p_all[:, t, c * 512:(c + 1) * 512],
                             start=(t == 0), stop=(t == T - 1))

    # ---- epilogue: leakyrelu on ACT straight from PSUM, DMA out transposed ----
    for c in range(NC):
        sl = slice(c * 512, (c + 1) * 512)
        nc.scalar.activation(o_sb[:, sl], hp_ps[c][:], AF.Prelu,
                             bias=0.0, scale=1.0, alpha=NEG_SLOPE)
        nc.sync.dma_start(out[:, sl], o_sb[:, sl])


_NC_CACHE = {}


def _build_nc():
    if "nc" in _NC_CACHE:
        return _NC_CACHE["nc"]
    nc = bacc.Bacc("TRN2", target_bir_lowering=False, debug=False)
    x = nc.dram_tensor("x", (N, FIN), F32, kind="ExternalInput").ap()
    w = nc.dram_tensor("w", (FIN, FOUT), F32, kind="ExternalInput").ap()
    a = nc.dram_tensor("a", (2 * FOUT, 1), F32, kind="ExternalInput").ap()
    # transposed output; the host un-transposes
    out = nc.dram_tensor("out", (FOUT, N), F32, kind="ExternalOutput").ap()
    with tile.TileContext(nc) as tc:
        _gat_body(tc, x, w, a, out)
    nc.compile()
    _NC_CACHE["nc"] = nc
    return nc


def kernel(x, W, a):
    x = np.ascontiguousarray(np.asarray(x), dtype=np.float32)
    W = np.ascontiguousarray(np.asarray(W), dtype=np.float32)
    a = np.ascontiguousarray(np.asarray(a), dtype=np.float32)
    assert x.shape == (N_CORES, N, FIN), x.shape
    nc = _build_nc()
    in_maps = [{"x": x[c], "w": W, "a": a} for c in range(N_CORES)]
    res = run_bass_kernel_spmd(nc, in_maps, core_ids=list(range(N_CORES)))
    return np.stack([res.results[c]["out"].T.copy() for c in range(N_CORES)], axis=0)
